# revision 4
# baseline (speedup 1.0000x reference)
"""AdaptiveLayerNorm Trainium2 kernel (8-core SPMD, data-parallel over tokens).

out = sigmoid(LN_w(s) @ W_s.T + b_s) * LN(a) + LN_w(s) @ W_nb.T

Sharding: tokens (B*N = 32768) split evenly across 8 cores; weights replicated.
No collectives needed.
"""

import sys
import os

sys.path.insert(0, "/opt/trn_rl_repo")

import numpy as np
import ml_dtypes

# Problem constants (hardcoded per harness contract)
B, N, CA, CS = 4, 8192, 768, 384
NCORES = 8
TOK = B * N                    # 32768
TPC = TOK // NCORES            # 4096 tokens per core
P = 128                        # partitions / tokens per tile
NTILES = TPC // P              # 32
EPS = 1e-5

_BUILD_CACHE = {}


def _build_graph():
    """Build the Bacc graph (single SPMD program, same for all cores)."""
    import concourse.bass as bass
    import concourse.tile as tile
    from concourse import bacc, mybir

    dt = mybir.dt

    nc = bacc.Bacc(
        "TRN2",
        target_bir_lowering=False,
        debug=False,
        num_devices=NCORES,
    )

    a_d = nc.dram_tensor("a", [TPC, CA], dt.float32, kind="ExternalInput").ap()
    s_d = nc.dram_tensor("s", [TPC, CS], dt.float32, kind="ExternalInput").ap()
    # WcatT = concat([W_s*ln_w, W_nb*ln_w], axis=0).T  -> [CS, 2*CA], bf16
    w_d = nc.dram_tensor("wcat", [CS, 2 * CA], dt.bfloat16, kind="ExternalInput").ap()
    # b_s broadcast to [P, CA] f32
    bb_d = nc.dram_tensor("bbias", [P, CA], dt.float32, kind="ExternalInput").ap()
    id_d = nc.dram_tensor("ident", [P, P], dt.bfloat16, kind="ExternalInput").ap()
    out_d = nc.dram_tensor("out", [TPC, CA], dt.float32, kind="ExternalOutput").ap()

    KC = CS // P  # 3 contraction chunks

    with tile.TileContext(nc) as tc:
        from contextlib import ExitStack

        with ExitStack() as ctx:
            const = ctx.enter_context(tc.tile_pool(name="const", bufs=1))
            io = ctx.enter_context(tc.tile_pool(name="io", bufs=3))
            wp = ctx.enter_context(tc.tile_pool(name="wp", bufs=3))
            stat = ctx.enter_context(tc.tile_pool(name="stat", bufs=4))
            pst = ctx.enter_context(tc.tile_pool(name="pst", bufs=2, space="PSUM"))
            pmm = ctx.enter_context(tc.tile_pool(name="pmm", bufs=2, space="PSUM"))

            # ---- constants, loaded once ----
            w_sb = const.tile([P, KC, 2 * CA], dt.bfloat16)
            for k in range(KC):
                nc.sync.dma_start(out=w_sb[:, k, :], in_=w_d[k * P : (k + 1) * P, :])
            bb_sb = const.tile([P, CA], dt.float32)
            nc.sync.dma_start(out=bb_sb[:], in_=bb_d[:, :])
            ident = const.tile([P, P], dt.bfloat16)
            nc.sync.dma_start(out=ident[:], in_=id_d[:, :])
            eps_t = const.tile([P, 1], dt.float32)
            nc.vector.memset(eps_t[:], EPS)

            for i in range(NTILES):
                r0 = i * P
                # ---- loads ----
                a_t = io.tile([P, CA], dt.float32)
                nc.sync.dma_start(out=a_t[:], in_=a_d[r0 : r0 + P, :])
                s_t = io.tile([P, CS], dt.float32)
                nc.sync.dma_start(out=s_t[:], in_=s_d[r0 : r0 + P, :])

                # ---- s stats ----
                st6_s = stat.tile([P, 6], dt.float32)
                nc.vector.bn_stats(st6_s[:], s_t[:])
                mv_s = stat.tile([P, 2], dt.float32)
                nc.vector.bn_aggr(mv_s[:], st6_s[:])
                inv_s = stat.tile([P, 1], dt.float32)
                # sqrt(var + eps) then reciprocal
                nc.scalar.activation(
                    out=inv_s[:],
                    in_=mv_s[:, 1:2],
                    func=mybir.ActivationFunctionType.Sqrt,
                    bias=eps_t[:],
                    scale=1.0,
                )
                nc.vector.reciprocal(out=inv_s[:], in_=inv_s[:])

                # ---- normalize s -> bf16 ----
                s_hat = wp.tile([P, CS], dt.bfloat16)
                nc.vector.tensor_scalar(
                    out=s_hat[:],
                    in0=s_t[:],
                    scalar1=mv_s[:, 0:1],
                    scalar2=inv_s[:],
                    op0=mybir.AluOpType.subtract,
                    op1=mybir.AluOpType.mult,
                )

                # ---- transpose s_hat (PE) ----
                psT = pst.tile([P, KC, P], dt.bfloat16)
                for k in range(KC):
                    nc.tensor.transpose(
                        psT[:, k, :], s_hat[:, k * P : (k + 1) * P], ident[:]
                    )
                sT = wp.tile([P, KC, P], dt.bfloat16)
                nc.scalar.copy(out=sT[:], in_=psT[:])

                # ---- matmul: [P,1536] += s_hatT.T @ WcatT ----
                mm = pmm.tile([P, 2 * CA], dt.float32)
                for n in range(3):
                    for k in range(KC):
                        nc.tensor.matmul(
                            mm[:, n * 512 : (n + 1) * 512],
                            lhsT=sT[:, k, :],
                            rhs=w_sb[:, k, n * 512 : (n + 1) * 512],
                            start=(k == 0),
                            stop=(k == KC - 1),
                        )

                # ---- a stats ----
                st6_a = stat.tile([P, 2, 6], dt.float32)
                a_r = a_t[:].rearrange("p (n d) -> p n d", n=2)
                for h in range(2):
                    nc.vector.bn_stats(st6_a[:, h, :], a_r[:, h, :])
                mv_a = stat.tile([P, 2], dt.float32)
                nc.vector.bn_aggr(mv_a[:], st6_a[:])
                inv_a = stat.tile([P, 1], dt.float32)
                nc.scalar.activation(
                    out=inv_a[:],
                    in_=mv_a[:, 1:2],
                    func=mybir.ActivationFunctionType.Sqrt,
                    bias=eps_t[:],
                    scale=1.0,
                )
                nc.vector.reciprocal(out=inv_a[:], in_=inv_a[:])

                # ---- gate = sigmoid(mm[:, :CA] + b) ----
                gb = wp.tile([P, CA], dt.float32)
                nc.vector.tensor_add(out=gb[:], in0=mm[:, 0:CA], in1=bb_sb[:])
                g = wp.tile([P, CA], dt.float32)
                nc.scalar.activation(
                    out=g[:], in_=gb[:], func=mybir.ActivationFunctionType.Sigmoid
                )

                # ---- out = ((a - mu_a) * g) * inv_a + skip ----
                m = wp.tile([P, CA], dt.float32)
                nc.vector.scalar_tensor_tensor(
                    out=m[:],
                    in0=a_t[:],
                    scalar=mv_a[:, 0:1],
                    in1=g[:],
                    op0=mybir.AluOpType.subtract,
                    op1=mybir.AluOpType.mult,
                )
                o_t = io.tile([P, CA], dt.float32)
                nc.vector.scalar_tensor_tensor(
                    out=o_t[:],
                    in0=m[:],
                    scalar=inv_a[:],
                    in1=mm[:, CA : 2 * CA],
                    op0=mybir.AluOpType.mult,
                    op1=mybir.AluOpType.add,
                )
                nc.sync.dma_start(out=out_d[r0 : r0 + P, :], in_=o_t[:])

    nc.compile()
    return nc


def _get_graph():
    if "nc" not in _BUILD_CACHE:
        _BUILD_CACHE["nc"] = _build_graph()
    return _BUILD_CACHE["nc"]


def _host_prep(a, s, ln_s_w, W_s, b_s, W_nb):
    """Shard inputs and prepare derived weights."""
    bf16 = ml_dtypes.bfloat16
    a2 = np.ascontiguousarray(a.reshape(TOK, CA))
    s2 = np.ascontiguousarray(s.reshape(TOK, CS))

    wg = (W_s * ln_s_w[None, :]).astype(np.float32)      # [CA, CS]
    wk = (W_nb * ln_s_w[None, :]).astype(np.float32)     # [CA, CS]
    wcat = np.concatenate([wg, wk], axis=0)              # [2CA, CS]
    wcatT = np.ascontiguousarray(wcat.T).astype(bf16)    # [CS, 2CA]
    bb = np.ascontiguousarray(np.tile(b_s[None, :].astype(np.float32), (P, 1)))
    ident = np.eye(P, dtype=bf16)

    in_maps = []
    for c in range(NCORES):
        in_maps.append(
            {
                "a": np.ascontiguousarray(a2[c * TPC : (c + 1) * TPC]),
                "s": np.ascontiguousarray(s2[c * TPC : (c + 1) * TPC]),
                "wcat": wcatT,
                "bbias": bb,
                "ident": ident,
            }
        )
    return in_maps


def _install_ntff_hook():
    """Register the axon NTFF profile hook that the container's antenv stub lacks."""
    import types
    import antenv

    if "antenv.axon_hooks" not in sys.modules:
        mod = types.ModuleType("antenv.axon_hooks")
        mod._hook = None

        def set_axon_ntff_profile_hook(h):
            mod._hook = h

        def get_axon_ntff_profile_hook():
            return mod._hook

        mod.set_axon_ntff_profile_hook = set_axon_ntff_profile_hook
        mod.get_axon_ntff_profile_hook = get_axon_ntff_profile_hook
        sys.modules["antenv.axon_hooks"] = mod
        antenv.axon_hooks = mod

    hooks = sys.modules["antenv.axon_hooks"]
    if hooks._hook is None:
        from trn_agent_boot.trn_boot import _ntff_profile_via_ctypes

        hooks.set_axon_ntff_profile_hook(
            _ntff_profile_via_ctypes("/opt/axon/libaxon_pjrt.so")
        )

    # upload_artifacts needs external bucket access; stub it out.
    from concourse import bass_utils

    bass_utils.upload_artifacts = lambda tmpdir: f"local:{tmpdir}"


def run(inputs, trace=False):
    """Run on 8 NeuronCores. Returns (out_full [B,N,CA] f32, exec_time_ns|None)."""
    from concourse.bass_utils import run_bass_kernel_spmd

    if trace:
        _install_ntff_hook()
    nc = _get_graph()
    in_maps = _host_prep(**inputs)
    res = run_bass_kernel_spmd(
        nc, in_maps, core_ids=list(range(NCORES)), trace=trace
    )
    outs = [np.asarray(res.results[c]["out"], dtype=np.float32) for c in range(NCORES)]
    full = np.concatenate(outs, axis=0).reshape(B, N, CA)
    return full, res.exec_time_ns


def kernel(**inputs):
    out, _ = run(inputs, trace=False)
    return out
